# revision 13
# baseline (speedup 1.0000x reference)
"""Trainium2 Bass kernel for nn_DGG_StraightThrough.

The reference's pairwise-logit MLP is mathematically dead: softmax over the
singleton feature dim is identically 1, so log_p == 0 and the gumbel logits
y equal `temp` exactly.  adj[b,i,j] = 1.0 iff temp[i,j] is among the 8
largest of row i, identical across the batch.

Sharding: row-parallel over N=2048 across 8 cores (256 rows/core, two
128-row chunks living side by side in one [128, 4096] SBUF tile).

v10 (vs the 23.9us single-queue baseline):
  - Both HWDGE queues (sync + scalar) co-stream chunk0's column halves
    first, then chunk1's.  Hierarchical MAX8 (one per queue piece +
    16->8 merge) starts as soon as the FASTER queue's half lands -- the
    queues' ring start order flip-flops run to run, so this decouples
    DVE start from the slower one.
  - The mask compares move off DVE: chunk0 and chunk1's column half b
    run on the Scalar engine as u8(Sign(t - thr + 1e-6)) (saturating
    cast maps sign's -1 to 0; +1e-6 = 2 ulps keeps the threshold
    element at 1; the false-positive window is ~5e-5 per core, far
    inside the 2e-2 budget).  A dummy Sign at block entry prewarms the
    1.3us activation function table during the DMA stream phase.
    Chunk1's compare is split DVE-is_ge / Scalar-Sign, balanced
    1178/870 by engine rate, and each half's output DMA is issued the
    moment that half's mask is ready (sync ships DVE's half, scalar
    ships its own), so the out tail is two overlapped ~128KB streams.
  - Chunk1 streams as four 512-column pieces (two per queue) whose
    MAX8s chase the landings, leaving only a 0.6us piece-max after the
    last landing instead of a 1.2us half-max.

Host: concatenate 8 u8 slabs, cast to f32, broadcast over B=4.
"""

import sys

import numpy as np

if "/opt/trn_rl_repo" not in sys.path:
    sys.path.insert(0, "/opt/trn_rl_repo")

B, N, K = 4, 2048, 8
N_CORES = 8
ROWS = N // N_CORES  # 256 rows per core
P = 128  # SBUF partitions
H = N // 2  # column half per queue
Q = N // 4  # chunk1 stream piece
# balanced chunk1 compare split: DVE is_ge runs ~0.73ns/col, scalar Sign
# ~0.98ns/col, so DVE takes 1178 columns and scalar 870
CMP = 1178

# Hooks for a driving harness (test.py): extra kwargs for run_bass_kernel_spmd
# and the last BassKernelResults (exec_time_ns etc).
RUN_KWARGS: dict = {}
LAST_RESULT = None

_PROGRAM = None


def _build_program():
    import concourse.bass as bass
    import concourse.mybir as mybir

    class _LeanBass(bass.Bass):
        # Skip the barrier Bass.__init__ emits after const-AP registration:
        # this kernel never reads const APs, and Sync's DGE table load
        # precedes its DMAs in program order.  Saves ~1us of preamble.
        _skip_init_barrier = False

        def all_engine_barrier(self, **kw):
            if _LeanBass._skip_init_barrier:
                return
            return super().all_engine_barrier(**kw)

    _LeanBass._skip_init_barrier = True
    try:
        nc = _LeanBass(enable_partition_id=False, monotonic_sem_count=0)
    finally:
        _LeanBass._skip_init_barrier = False
    t_in = nc.declare_dram_parameter("t", [ROWS, N], mybir.dt.float32, isOutput=False)
    # u8 wire format for the 0/1 mask (lossless); host casts back to f32
    out = nc.declare_dram_parameter("out", [ROWS, N], mybir.dt.uint8, isOutput=True)

    AF = mybir.ActivationFunctionType

    with (
        nc.sbuf_tensor([P, 2 * N], mybir.dt.float32) as tile,
        nc.sbuf_tensor([P, 2 * N], mybir.dt.uint8) as mask,
        # chunk0: 2 piece top8s + merged; chunk1: 4 piece top8s + merged
        nc.sbuf_tensor([P, 64], mybir.dt.float32) as t8,
        nc.sbuf_tensor([P, 2], mybir.dt.float32) as neg,
        nc.sbuf_tensor([P, 4], mybir.dt.uint8) as scr8,
        # per-transfer in-DMA sems: transfers on different queues complete
        # out of order, so shared counting sems would race
        nc.semaphore("in_a0") as in_a0,
        nc.semaphore("in_b0") as in_b0,
        nc.semaphore("in_a1") as in_a1,
        nc.semaphore("in_a2") as in_a2,
        nc.semaphore("in_b1") as in_b1,
        nc.semaphore("in_b2") as in_b2,
        nc.semaphore("v_sem") as v_sem,
        nc.semaphore("s_sem") as s_sem,
        nc.semaphore("out_sem") as out_sem,
    ):
        # In-DMAs issued OUTSIDE the Block, right after each queue engine's
        # DGE-table preamble.  Chunk0's column halves go first on both
        # queues (4KB descriptors), then chunk1's.
        nc.sync.dma_start(out=tile[:, 0:H], in_=t_in[0:P, 0:H]).then_inc(in_a0, 16)
        nc.scalar.dma_start(out=tile[:, H:N], in_=t_in[0:P, H:N]).then_inc(in_b0, 16)
        nc.sync.dma_start(
            out=tile[:, N : N + Q], in_=t_in[P : 2 * P, 0:Q]
        ).then_inc(in_a1, 16)
        nc.sync.dma_start(
            out=tile[:, N + Q : N + H], in_=t_in[P : 2 * P, Q:H]
        ).then_inc(in_a2, 16)
        nc.scalar.dma_start(
            out=tile[:, N + H : N + H + Q], in_=t_in[P : 2 * P, H : H + Q]
        ).then_inc(in_b1, 16)
        nc.scalar.dma_start(
            out=tile[:, N + H + Q : 2 * N], in_=t_in[P : 2 * P, H + Q : N]
        ).then_inc(in_b2, 16)

        # no SWDGE DMAs issued -> skip GpSimd's expensive dge_drain at exit
        with nc.Block(no_gpsimd_drain=True) as block:

            @block.vector
            def _(vector):
                # Hierarchical MAX8: one per landed queue piece, then a
                # 16->8 merge (exact: any row-top-8 element is in its
                # piece's top-8).  neg = -thr + 1e-6 feeds the scalar
                # engine's Sign compare.  Sem self-hops guard same-engine
                # RAWs on t8 (stream-read / scalar-ptr fetch race the
                # in-pipeline write of the previous op).
                # v_sem: 1=mA0 2=mB0 3=mrg0 4=neg0 5-8=c1 piece maxes
                #        9=mrg1 10=neg1 11=cmp1a
                vector.wait_ge(in_a0, 16)
                vector.max(t8[:, 0:8], tile[:, 0:H]).then_inc(v_sem, 1)
                vector.wait_ge(in_b0, 16)
                vector.max(t8[:, 8:16], tile[:, H:N]).then_inc(v_sem, 1)
                vector.wait_ge(v_sem, 2)
                vector.max(t8[:, 16:24], t8[:, 0:16]).then_inc(v_sem, 1)
                vector.wait_ge(v_sem, 3)
                vector.tensor_scalar(
                    neg[:, 0:1],
                    t8[:, 23:24],
                    -1.0,
                    1e-6,
                    mybir.AluOpType.mult,
                    mybir.AluOpType.add,
                ).then_inc(v_sem, 1)
                # chunk1 piece maxes in per-queue landing order: the sync
                # queue's two pieces land in order, then the scalar queue's
                # (which landed meanwhile on a balanced run) -- robust to
                # either queue drawing the slow ring start
                vector.wait_ge(in_a1, 16)
                vector.max(t8[:, 24:32], tile[:, N : N + Q]).then_inc(v_sem, 1)
                vector.wait_ge(in_a2, 16)
                vector.max(
                    t8[:, 32:40], tile[:, N + Q : N + H]
                ).then_inc(v_sem, 1)
                vector.wait_ge(in_b1, 16)
                vector.max(
                    t8[:, 40:48], tile[:, N + H : N + H + Q]
                ).then_inc(v_sem, 1)
                vector.wait_ge(in_b2, 16)
                vector.max(
                    t8[:, 48:56], tile[:, N + H + Q : 2 * N]
                ).then_inc(v_sem, 1)
                vector.wait_ge(v_sem, 8)
                vector.max(t8[:, 56:64], t8[:, 24:56]).then_inc(v_sem, 1)
                vector.wait_ge(v_sem, 9)
                vector.tensor_scalar(
                    neg[:, 1:2],
                    t8[:, 63:64],
                    -1.0,
                    1e-6,
                    mybir.AluOpType.mult,
                    mybir.AluOpType.add,
                ).then_inc(v_sem, 1)
                # chunk1 compare, DVE's exact-is_ge share
                vector.tensor_scalar(
                    mask[:, N : N + CMP],
                    tile[:, N : N + CMP],
                    t8[:, 63:64],
                    None,
                    mybir.AluOpType.is_ge,
                ).then_inc(v_sem, 1)

            @block.scalar
            def _(scalar):
                # Dummy Sign on scratch: pulls the 1.3us activation
                # function-table load into the DMA stream phase.
                scalar.activation(
                    scr8[:, 0:1],
                    neg[:, 0:1],
                    AF.Sign,
                    bias=neg[:, 1:2],
                    scale=1.0,
                )
                # chunk0 compare: u8(Sign(t - thr + 1e-6)) == (t >= thr)
                scalar.wait_ge(v_sem, 4)
                scalar.activation(
                    mask[:, 0:N],
                    tile[:, 0:N],
                    AF.Sign,
                    bias=neg[:, 0:1],
                    scale=1.0,
                ).then_inc(s_sem, 1)
                # chunk1 compare, scalar's share; it then ships its own
                # half the moment the Sign completes (self-hop on s_sem so
                # the DMA cannot read mask before the writes commit)
                scalar.wait_ge(v_sem, 10)
                scalar.activation(
                    mask[:, N + CMP : 2 * N],
                    tile[:, N + CMP : 2 * N],
                    AF.Sign,
                    bias=neg[:, 1:2],
                    scale=1.0,
                ).then_inc(s_sem, 1)
                scalar.wait_ge(s_sem, 2)
                scalar.dma_start(
                    out=out[P : 2 * P, CMP:N], in_=mask[:, N + CMP : 2 * N]
                ).then_inc(out_sem, 16)

            @block.sync
            def _(sync):
                sync.wait_ge(s_sem, 1)
                sync.dma_start(out=out[0:P, :], in_=mask[:, 0:N]).then_inc(out_sem, 16)
                sync.wait_ge(v_sem, 11)
                sync.dma_start(
                    out=out[P : 2 * P, 0:CMP], in_=mask[:, N : N + CMP]
                ).then_inc(out_sem, 16)
                sync.wait_ge(out_sem, 48)

    return nc


def kernel(**inputs: np.ndarray) -> np.ndarray:
    global _PROGRAM, LAST_RESULT
    from concourse.bass_utils import run_bass_kernel_spmd

    temp = np.ascontiguousarray(np.asarray(inputs["temp"], dtype=np.float32))
    assert temp.shape == (N, N)

    in_maps = [
        {"t": np.ascontiguousarray(temp[c * ROWS : (c + 1) * ROWS])}
        for c in range(N_CORES)
    ]

    res = None
    last_err = None
    for attempt in range(3):
        try:
            if _PROGRAM is None:
                _PROGRAM = _build_program()
            res = run_bass_kernel_spmd(
                _PROGRAM, in_maps, list(range(N_CORES)), **RUN_KWARGS
            )
            break
        except Exception as e:  # transient device wedges (e.g. NRT unrecoverable)
            last_err = e
            _PROGRAM = None
            if attempt == 2:
                raise
            import time

            time.sleep(10 * (attempt + 1))
            try:  # recreate the PJRT client, as a fresh process would
                import jax

                jax.clear_backends()
                jax.devices()
            except Exception:
                pass
    assert res is not None, last_err
    LAST_RESULT = res

    mask = np.concatenate([res.results[c]["out"] for c in range(N_CORES)], axis=0)
    mask = mask.astype(np.float32)
    return np.ascontiguousarray(np.broadcast_to(mask[None], (B, N, N)))


# revision 14
# speedup vs baseline: 1.0174x; 1.0174x over previous
"""Trainium2 Bass kernel for nn_DGG_StraightThrough.

The reference's pairwise-logit MLP is mathematically dead: softmax over the
singleton feature dim is identically 1, so log_p == 0 and the gumbel logits
y equal `temp` exactly.  adj[b,i,j] = 1.0 iff temp[i,j] is among the 8
largest of row i, identical across the batch.

Sharding: row-parallel over N=2048 across 8 cores (256 rows/core, two
128-row chunks living side by side in one [128, 4096] SBUF tile).

v11 (vs the 23.9us single-queue baseline):
  - Both HWDGE queues (sync + scalar) co-stream chunk0's column halves
    first, then chunk1's.  Hierarchical MAX8 (one per queue piece +
    16->8 merge) starts as soon as the FASTER queue's half lands -- the
    queues' ring start order flip-flops run to run, so this decouples
    DVE start from the slower one.
  - The mask compares move off DVE: chunk0 and chunk1's column half b
    run on the Scalar engine as u8(Sign(t - thr + 1e-6)) (saturating
    cast maps sign's -1 to 0; +1e-6 = 2 ulps keeps the threshold
    element at 1; the false-positive window is ~5e-5 per core, far
    inside the 2e-2 budget).  A dummy Sign at block entry prewarms the
    1.3us activation function table during the DMA stream phase.
    Chunk1's compare is split DVE-is_ge / Scalar-Sign, balanced
    1178/870 by engine rate, and each half's output DMA is issued the
    moment that half's mask is ready (sync ships DVE's half, scalar
    ships its own), so the out tail is two overlapped ~128KB streams.
  - BOTH chunks stream as four 512-column pieces (two per queue) whose
    MAX8s chase the landings, so the serial DVE chain rides the whole
    stream: after the last landing only a 0.6us piece-max + merge +
    compare + out remain, regardless of how slow the (HBM-contended)
    stream phase runs.

Host: concatenate 8 u8 slabs, cast to f32, broadcast over B=4.
"""

import sys

import numpy as np

if "/opt/trn_rl_repo" not in sys.path:
    sys.path.insert(0, "/opt/trn_rl_repo")

B, N, K = 4, 2048, 8
N_CORES = 8
ROWS = N // N_CORES  # 256 rows per core
P = 128  # SBUF partitions
H = N // 2  # column half per queue
Q = N // 4  # stream piece columns
# balanced chunk1 compare split: DVE is_ge runs ~0.73ns/col, scalar Sign
# ~0.98ns/col, so DVE takes 1178 columns and scalar 870
CMP = 1178

# Hooks for a driving harness (test.py): extra kwargs for run_bass_kernel_spmd
# and the last BassKernelResults (exec_time_ns etc).
RUN_KWARGS: dict = {}
LAST_RESULT = None

_PROGRAM = None


def _build_program():
    import concourse.bass as bass
    import concourse.mybir as mybir

    class _LeanBass(bass.Bass):
        # Skip the barrier Bass.__init__ emits after const-AP registration:
        # this kernel never reads const APs, and Sync's DGE table load
        # precedes its DMAs in program order.  Saves ~1us of preamble.
        _skip_init_barrier = False

        def all_engine_barrier(self, **kw):
            if _LeanBass._skip_init_barrier:
                return
            return super().all_engine_barrier(**kw)

    _LeanBass._skip_init_barrier = True
    try:
        nc = _LeanBass(enable_partition_id=False, monotonic_sem_count=0)
    finally:
        _LeanBass._skip_init_barrier = False
    t_in = nc.declare_dram_parameter("t", [ROWS, N], mybir.dt.float32, isOutput=False)
    # u8 wire format for the 0/1 mask (lossless); host casts back to f32
    out = nc.declare_dram_parameter("out", [ROWS, N], mybir.dt.uint8, isOutput=True)

    AF = mybir.ActivationFunctionType

    with (
        nc.sbuf_tensor([P, 2 * N], mybir.dt.float32) as tile,
        nc.sbuf_tensor([P, 2 * N], mybir.dt.uint8) as mask,
        # per chunk: four piece top8s + the merged top8
        nc.sbuf_tensor([P, 80], mybir.dt.float32) as t8,
        nc.sbuf_tensor([P, 2], mybir.dt.float32) as neg,
        nc.sbuf_tensor([P, 4], mybir.dt.uint8) as scr8,
        # per-transfer in-DMA sems: transfers on different queues complete
        # out of order, so shared counting sems would race
        nc.semaphore("in_a0") as in_a0,
        nc.semaphore("in_a0b") as in_a0b,
        nc.semaphore("in_b0") as in_b0,
        nc.semaphore("in_b0b") as in_b0b,
        nc.semaphore("in_a1") as in_a1,
        nc.semaphore("in_a2") as in_a2,
        nc.semaphore("in_b1") as in_b1,
        nc.semaphore("in_b2") as in_b2,
        nc.semaphore("v_sem") as v_sem,
        nc.semaphore("s_sem") as s_sem,
        nc.semaphore("out_sem") as out_sem,
    ):
        # In-DMAs issued OUTSIDE the Block, right after each queue engine's
        # DGE-table preamble.  Chunk0's four pieces go first (two per
        # queue), then chunk1's four.
        nc.sync.dma_start(out=tile[:, 0:Q], in_=t_in[0:P, 0:Q]).then_inc(in_a0, 16)
        nc.sync.dma_start(out=tile[:, Q:H], in_=t_in[0:P, Q:H]).then_inc(in_a0b, 16)
        nc.scalar.dma_start(
            out=tile[:, H : H + Q], in_=t_in[0:P, H : H + Q]
        ).then_inc(in_b0, 16)
        nc.scalar.dma_start(
            out=tile[:, H + Q : N], in_=t_in[0:P, H + Q : N]
        ).then_inc(in_b0b, 16)
        nc.sync.dma_start(
            out=tile[:, N : N + Q], in_=t_in[P : 2 * P, 0:Q]
        ).then_inc(in_a1, 16)
        nc.sync.dma_start(
            out=tile[:, N + Q : N + H], in_=t_in[P : 2 * P, Q:H]
        ).then_inc(in_a2, 16)
        nc.scalar.dma_start(
            out=tile[:, N + H : N + H + Q], in_=t_in[P : 2 * P, H : H + Q]
        ).then_inc(in_b1, 16)
        nc.scalar.dma_start(
            out=tile[:, N + H + Q : 2 * N], in_=t_in[P : 2 * P, H + Q : N]
        ).then_inc(in_b2, 16)

        # no SWDGE DMAs issued -> skip GpSimd's expensive dge_drain at exit
        with nc.Block(no_gpsimd_drain=True) as block:

            @block.vector
            def _(vector):
                # Hierarchical MAX8: one per landed queue piece, then a
                # 16->8 merge (exact: any row-top-8 element is in its
                # piece's top-8).  neg = -thr + 1e-6 feeds the scalar
                # engine's Sign compare.  Sem self-hops guard same-engine
                # RAWs on t8 (stream-read / scalar-ptr fetch race the
                # in-pipeline write of the previous op).
                # v_sem: 1-4=c0 piece maxes 5=mrg0 6=neg0
                #        7-10=c1 piece maxes 11=mrg1 12=neg1 13=cmp1a
                vector.wait_ge(in_a0, 16)
                vector.max(t8[:, 0:8], tile[:, 0:Q]).then_inc(v_sem, 1)
                vector.wait_ge(in_a0b, 16)
                vector.max(t8[:, 8:16], tile[:, Q:H]).then_inc(v_sem, 1)
                vector.wait_ge(in_b0, 16)
                vector.max(t8[:, 16:24], tile[:, H : H + Q]).then_inc(v_sem, 1)
                vector.wait_ge(in_b0b, 16)
                vector.max(t8[:, 24:32], tile[:, H + Q : N]).then_inc(v_sem, 1)
                vector.wait_ge(v_sem, 4)
                vector.max(t8[:, 32:40], t8[:, 0:32]).then_inc(v_sem, 1)
                vector.wait_ge(v_sem, 5)
                vector.tensor_scalar(
                    neg[:, 0:1],
                    t8[:, 39:40],
                    -1.0,
                    1e-6,
                    mybir.AluOpType.mult,
                    mybir.AluOpType.add,
                ).then_inc(v_sem, 1)
                # chunk1 piece maxes in per-queue landing order: the sync
                # queue's two pieces land in order, then the scalar queue's
                # (which landed meanwhile on a balanced run) -- robust to
                # either queue drawing the slow ring start
                vector.wait_ge(in_a1, 16)
                vector.max(t8[:, 40:48], tile[:, N : N + Q]).then_inc(v_sem, 1)
                vector.wait_ge(in_a2, 16)
                vector.max(
                    t8[:, 48:56], tile[:, N + Q : N + H]
                ).then_inc(v_sem, 1)
                vector.wait_ge(in_b1, 16)
                vector.max(
                    t8[:, 56:64], tile[:, N + H : N + H + Q]
                ).then_inc(v_sem, 1)
                vector.wait_ge(in_b2, 16)
                vector.max(
                    t8[:, 64:72], tile[:, N + H + Q : 2 * N]
                ).then_inc(v_sem, 1)
                vector.wait_ge(v_sem, 10)
                vector.max(t8[:, 72:80], t8[:, 40:72]).then_inc(v_sem, 1)
                vector.wait_ge(v_sem, 11)
                vector.tensor_scalar(
                    neg[:, 1:2],
                    t8[:, 79:80],
                    -1.0,
                    1e-6,
                    mybir.AluOpType.mult,
                    mybir.AluOpType.add,
                ).then_inc(v_sem, 1)
                # chunk1 compare, DVE's exact-is_ge share
                vector.tensor_scalar(
                    mask[:, N : N + CMP],
                    tile[:, N : N + CMP],
                    t8[:, 79:80],
                    None,
                    mybir.AluOpType.is_ge,
                ).then_inc(v_sem, 1)

            @block.scalar
            def _(scalar):
                # Dummy Sign on scratch: pulls the 1.3us activation
                # function-table load into the DMA stream phase.
                scalar.activation(
                    scr8[:, 0:1],
                    neg[:, 0:1],
                    AF.Sign,
                    bias=neg[:, 1:2],
                    scale=1.0,
                )
                # chunk0 compare: u8(Sign(t - thr + 1e-6)) == (t >= thr)
                scalar.wait_ge(v_sem, 6)
                scalar.activation(
                    mask[:, 0:N],
                    tile[:, 0:N],
                    AF.Sign,
                    bias=neg[:, 0:1],
                    scale=1.0,
                ).then_inc(s_sem, 1)
                # chunk1 compare, scalar's share; it then ships its own
                # half the moment the Sign completes (self-hop on s_sem so
                # the DMA cannot read mask before the writes commit)
                scalar.wait_ge(v_sem, 12)
                scalar.activation(
                    mask[:, N + CMP : 2 * N],
                    tile[:, N + CMP : 2 * N],
                    AF.Sign,
                    bias=neg[:, 1:2],
                    scale=1.0,
                ).then_inc(s_sem, 1)
                scalar.wait_ge(s_sem, 2)
                scalar.dma_start(
                    out=out[P : 2 * P, CMP:N], in_=mask[:, N + CMP : 2 * N]
                ).then_inc(out_sem, 16)

            @block.sync
            def _(sync):
                sync.wait_ge(s_sem, 1)
                sync.dma_start(out=out[0:P, :], in_=mask[:, 0:N]).then_inc(out_sem, 16)
                sync.wait_ge(v_sem, 13)
                sync.dma_start(
                    out=out[P : 2 * P, 0:CMP], in_=mask[:, N : N + CMP]
                ).then_inc(out_sem, 16)
                sync.wait_ge(out_sem, 48)

    return nc


def kernel(**inputs: np.ndarray) -> np.ndarray:
    global _PROGRAM, LAST_RESULT
    from concourse.bass_utils import run_bass_kernel_spmd

    temp = np.ascontiguousarray(np.asarray(inputs["temp"], dtype=np.float32))
    assert temp.shape == (N, N)

    in_maps = [
        {"t": np.ascontiguousarray(temp[c * ROWS : (c + 1) * ROWS])}
        for c in range(N_CORES)
    ]

    res = None
    last_err = None
    for attempt in range(3):
        try:
            if _PROGRAM is None:
                _PROGRAM = _build_program()
            res = run_bass_kernel_spmd(
                _PROGRAM, in_maps, list(range(N_CORES)), **RUN_KWARGS
            )
            break
        except Exception as e:  # transient device wedges (e.g. NRT unrecoverable)
            last_err = e
            _PROGRAM = None
            if attempt == 2:
                raise
            import time

            time.sleep(10 * (attempt + 1))
            try:  # recreate the PJRT client, as a fresh process would
                import jax

                jax.clear_backends()
                jax.devices()
            except Exception:
                pass
    assert res is not None, last_err
    LAST_RESULT = res

    mask = np.concatenate([res.results[c]["out"] for c in range(N_CORES)], axis=0)
    mask = mask.astype(np.float32)
    return np.ascontiguousarray(np.broadcast_to(mask[None], (B, N, N)))
